# revision 31
# baseline (speedup 1.0000x reference)
"""HFreqC layer kernel for 8 Trainium2 NeuronCores.

The reference op (FFT -> zero centered low-freq band -> IFFT -> real -> relu)
is, up to the relu, a fixed real linear operator along the channel axis:
    y = relu(x @ W),  W = Re(ifft(mask * fft(I)))^T   (728x728, symmetric)

Strategy: pure data parallel over rows (32*38*38 = 46208 rows). Each core
processes 46 row-tiles of 128 rows (5888 rows; last core is zero-padded).
All device I/O is bf16 (quantization adds ~0.2% rel err vs the 2e-2 gate),
which halves HBM traffic and makes the kernel TensorE-bound:
  - W (row-padded to 768) lives in SBUF as bf16 [128, 6*728].
  - Per row-tile one contiguous [128, 768] bf16 DMA holds the 6 k-tiles of
    X^T (channel-major: [t][p][u*128+m] = x[t*128+m, u*128+p]).
  - bf16 matmuls accumulate over 6 k-tiles into PSUM, j in two 364 chunks.
  - ScalarE applies relu on the PSUM->SBUF copy, casting to bf16.
  - One contiguous [128, 728] bf16 DMA out per row-tile.
Engine budget per row-tile: PE 12x364cyc ~ 1820ns (bound), SP in-DMA
~592ns, ACT 2 relus + out-DMA ~1540ns. Input prefetch stays on its own
queue (SP) so output DMAs can never head-of-line-block it.
"""

import numpy as np

C = 728            # channels
KT = 6             # k tiles of 128 (channel pad to 768)
CP = KT * 128      # 768 padded channels
N_CORES = 8
ROWS_TOTAL = 32 * 38 * 38          # 46208
N_TILES = 46                       # 128-row tiles per core
ROWS_PER_CORE = N_TILES * 128      # 5888 (padded; 8*5888 = 47104 >= 46208)
JC = 364           # j-chunk width (2 chunks of 364; psum bank holds 512 f32)

_CACHE = {}


def _f32_to_bf16_u16(a: np.ndarray) -> np.ndarray:
    """Round-to-nearest-even f32 -> bf16, as uint16 payload (fast, vectorized)."""
    u = a.view(np.uint32)
    rounded = u + np.uint32(0x7FFF) + ((u >> np.uint32(16)) & np.uint32(1))
    return (rounded >> np.uint32(16)).astype(np.uint16)


def _bf16_u16_to_f32(u: np.ndarray) -> np.ndarray:
    return (u.astype(np.uint32) << np.uint32(16)).view(np.float32)


def _bf16(a: np.ndarray):
    import ml_dtypes
    return _f32_to_bf16_u16(np.ascontiguousarray(a)).view(ml_dtypes.bfloat16)


def _build_w(scale: int) -> np.ndarray:
    """[CP, C] f32: W padded with zero rows; y_row = x_row @ W."""
    m_sh = np.ones(C)
    m_sh[C // 2 - C // scale: C // 2 + C // scale] = 0
    m = np.fft.ifftshift(m_sh)
    A = np.fft.ifft(m[:, None] * np.fft.fft(np.eye(C), axis=0), axis=0)
    W = np.real(A).T.astype(np.float32)
    Wp = np.zeros((CP, C), dtype=np.float32)
    Wp[:C] = W
    return Wp


def _shard_xt(xf: np.ndarray, core: int) -> np.ndarray:
    """[N_TILES, 128, CP] bf16: [t][p][u*128+m] = x[t*128+m, 128u+p]."""
    lo = core * ROWS_PER_CORE
    hi = min(lo + ROWS_PER_CORE, ROWS_TOTAL)
    xp = np.zeros((ROWS_PER_CORE, CP), dtype=np.uint16)
    xp[:hi - lo, :C] = _f32_to_bf16_u16(np.ascontiguousarray(xf[lo:hi]))
    v = xp.reshape(N_TILES, 128, KT, 128)              # t m u p
    v = v.transpose(0, 3, 2, 1)                        # t p u m
    import ml_dtypes
    return np.ascontiguousarray(v).reshape(N_TILES, 128, CP).view(ml_dtypes.bfloat16)


def _build_nc(repeat: int = 1, passes_per_iter: int = 1):
    """One full pass over the core's shard; repeat>1 wraps it in a HW loop
    (used only for steady-state timing -- same data is reprocessed;
    passes_per_iter unrolls extra passes inside the loop body)."""
    import concourse.mybir as mybir
    import concourse.tile as tile
    from concourse import bacc

    fp32 = mybir.dt.float32
    bf16 = mybir.dt.bfloat16

    nc = bacc.Bacc("TRN2", target_bir_lowering=False)
    x_d = nc.dram_tensor("x", [N_TILES, 128, CP], bf16, kind="ExternalInput").ap()
    w_d = nc.dram_tensor("w", [CP, C], bf16, kind="ExternalInput").ap()
    y_d = nc.dram_tensor("y", [N_TILES, 128, C], bf16, kind="ExternalOutput").ap()

    w_v = w_d.rearrange("(u p) j -> p u j", u=KT, p=128)

    with tile.TileContext(nc) as tc:
        with (
            tc.tile_pool(name="wpool", bufs=1) as wpool,
            tc.tile_pool(name="io", bufs=8) as io,
            tc.tile_pool(name="psp", bufs=6, space="PSUM") as psp,
        ):
            # Split the W load per k-tile so the first matmul only waits for
            # chunk u=0 (~2.3us) instead of the full 3.4us transfer.
            w_tiles = [wpool.tile([128, C], bf16, name=f"w{u}") for u in range(KT)]
            for u in range(KT):
                nc.scalar.dma_start(out=w_tiles[u], in_=w_v[:, u])

            def one_pass():
                for t in range(N_TILES):
                    xt = io.tile([128, CP], bf16, tag="xt")
                    nc.sync.dma_start(out=xt, in_=x_d[t])
                    ysb = io.tile([128, C], bf16, tag="y")
                    for jc in range(2):
                        j0 = jc * JC
                        ps = psp.tile([128, JC], fp32, tag="ps")
                        for u in range(KT):
                            nc.tensor.matmul(
                                ps,
                                lhsT=xt[:, u * 128:(u + 1) * 128],
                                rhs=w_tiles[u][:, j0:j0 + JC],
                                start=(u == 0),
                                stop=(u == KT - 1),
                            )
                        nc.scalar.activation(
                            ysb[:, j0:j0 + JC],
                            ps,
                            mybir.ActivationFunctionType.Relu,
                        )
                    nc.scalar.dma_start(out=y_d[t], in_=ysb)

            if repeat == 1:
                one_pass()
            else:
                import concourse.mybir as _mb
                with tc.For_i(0, repeat, 1,
                              hint_engines=(_mb.EngineType.PE,),
                              staggered_reset=True):
                    for _ in range(passes_per_iter):
                        one_pass()
    nc.compile()
    return nc


def _make_in_maps(x: np.ndarray, scale: int):
    xf = np.asarray(x, dtype=np.float32).reshape(-1, C)
    W = _bf16(_build_w(scale))
    return [{"x": _shard_xt(xf, i), "w": W} for i in range(N_CORES)]


def kernel(x: np.ndarray, scale) -> np.ndarray:
    import sys
    if "/opt/trn_rl_repo" not in sys.path:
        sys.path.insert(0, "/opt/trn_rl_repo")
    from concourse.bass_utils import run_bass_kernel_spmd

    scale = int(np.asarray(scale))
    x = np.asarray(x, dtype=np.float32)
    orig_shape = x.shape

    if "nc" not in _CACHE:
        _CACHE["nc"] = _build_nc()
    nc = _CACHE["nc"]

    in_maps = _make_in_maps(x, scale)
    res = run_bass_kernel_spmd(nc, in_maps, list(range(N_CORES)))
    outs = []
    for i, r in enumerate(res.results):
        lo = i * ROWS_PER_CORE
        hi = min(lo + ROWS_PER_CORE, ROWS_TOTAL)
        yb = np.asarray(r["y"]).reshape(ROWS_PER_CORE, C)[:hi - lo]
        outs.append(_bf16_u16_to_f32(yb.view(np.uint16)))
    y = np.concatenate(outs, axis=0).reshape(orig_shape)
    return y.astype(np.float32)



# revision 34
# speedup vs baseline: 1.0271x; 1.0271x over previous
"""HFreqC layer kernel for 8 Trainium2 NeuronCores.

The reference op (FFT -> zero centered low-freq band -> IFFT -> real -> relu)
is, up to the relu, a fixed real linear operator along the channel axis:
    y = x @ W,  W = Re(ifft(mask * fft(I)))^T   (728x728, symmetric circulant)

Key structure (scale=4 => the kept band is exactly half the spectrum): the
filter h(d) = (1/c) sum_{k in band} w^{kd} vanishes for all even lags d != 0
and h(0) = 1/2, so W couples only opposite parities plus a half-identity:
    y_even = relu(x_even/2 + x_odd  @ B1)   B1 = W[odd, even]  (364x364)
    y_odd  = relu(x_odd /2 + x_even @ B2)   B2 = W[even, odd]
This HALVES the matmul work vs the dense 728x728 GEMM.

Layout: pure data parallel over rows (46208 rows; 46 row-tiles of 128 per
core, last core zero-padded). All device I/O bf16 (~0.2% rel err vs the
2e-2 gate). Per row-tile:
  - [128, 768] bf16 channel-major DMA on SP: k-tiles 0-2 = odd channels
    (364 pad 384), 3-5 = even channels.
  - [128, 728] bf16 row-major x/2 DMA on GpSimd: [0:364]=x_even/2,
    [364:728]=x_odd/2 (host pre-scaled).
  - 3+3 bf16 matmuls accumulate B1/B2 products into two PSUM tiles.
  - DVE adds x/2 from SBUF onto each PSUM result (tensor_tensor) writing
    ysb in place; relu_even on ScalarE, relu_odd on DVE.
  - [128, 728] bf16 parity-ordered DMA out on ScalarE; host un-permutes.
Engine budget per row-tile (cost model): PE 6x364cyc ~ 910ns, DVE ~1140ns,
ACT ~1140ns, SP ~590ns, Pool ~560ns -> ~53us/core steady state.
"""

import numpy as np

C = 728            # channels
H = C // 2         # 364 per parity
KT = 3             # k-tiles of 128 per parity (364 pad 384)
CP = 2 * KT * 128  # 768 padded channels (odd block + even block)
N_CORES = 8
ROWS_TOTAL = 32 * 38 * 38          # 46208
N_TILES = 46                       # 128-row tiles per core
ROWS_PER_CORE = N_TILES * 128      # 5888 (padded; 8*5888 = 47104 >= 46208)

_CACHE = {}


def _f32_to_bf16_u16(a: np.ndarray) -> np.ndarray:
    """Round-to-nearest-even f32 -> bf16, as uint16 payload (fast, vectorized)."""
    u = np.ascontiguousarray(a).view(np.uint32)
    rounded = u + np.uint32(0x7FFF) + ((u >> np.uint32(16)) & np.uint32(1))
    return (rounded >> np.uint32(16)).astype(np.uint16)


def _bf16_u16_to_f32(u: np.ndarray) -> np.ndarray:
    return (u.astype(np.uint32) << np.uint32(16)).view(np.float32)


def _bf16(a: np.ndarray):
    import ml_dtypes
    return _f32_to_bf16_u16(np.ascontiguousarray(a)).view(ml_dtypes.bfloat16)


def _build_w(scale: int) -> np.ndarray:
    """Full [C, C] f32 W; y_row = x_row @ W."""
    m_sh = np.ones(C)
    m_sh[C // 2 - C // scale: C // 2 + C // scale] = 0
    m = np.fft.ifftshift(m_sh)
    A = np.fft.ifft(m[:, None] * np.fft.fft(np.eye(C), axis=0), axis=0)
    return np.real(A).T.astype(np.float32)


def _build_w_parity(scale: int) -> np.ndarray:
    """[2*KT, 128, H] bf16: tiles 0-2 = B1 = W[odd, even] rows (pad 384),
    tiles 3-5 = B2 = W[even, odd] rows. Asserts the parity structure."""
    W = _build_w(scale)
    d_e = W[0::2, 0::2]
    d_o = W[1::2, 1::2]
    assert np.abs(d_e - 0.5 * np.eye(H)).max() < 1e-5, "parity structure broken"
    assert np.abs(d_o - 0.5 * np.eye(H)).max() < 1e-5, "parity structure broken"
    B1 = W[1::2, 0::2]   # x_odd  -> y_even
    B2 = W[0::2, 1::2]   # x_even -> y_odd
    wp = np.zeros((2 * KT * 128, H), dtype=np.float32)
    wp[:H] = B1
    wp[KT * 128:KT * 128 + H] = B2
    return _bf16(wp.reshape(2 * KT, 128, H))


def _shard_xt(x16: np.ndarray, core: int) -> np.ndarray:
    """[N_TILES, 128, CP] bf16 channel-major: k-tiles 0-2 odd chs, 3-5 even.
    [t][p][u*128+m] = x[t*128+m, ch(u,p)]."""
    import ml_dtypes
    lo = core * ROWS_PER_CORE
    hi = min(lo + ROWS_PER_CORE, ROWS_TOTAL)
    xp = np.zeros((ROWS_PER_CORE, CP), dtype=np.uint16)
    xp[:hi - lo, :H] = x16[lo:hi, 1::2]            # odd channels
    xp[:hi - lo, KT * 128:KT * 128 + H] = x16[lo:hi, 0::2]  # even channels
    v = xp.reshape(N_TILES, 128, 2 * KT, 128)      # t m u p
    v = v.transpose(0, 3, 2, 1)                    # t p u m
    return np.ascontiguousarray(v).reshape(N_TILES, 128, CP).view(ml_dtypes.bfloat16)


def _shard_xr(xh16: np.ndarray, core: int) -> np.ndarray:
    """[N_TILES, 128, C] bf16 row-major x/2: [0:H]=x_even/2, [H:C]=x_odd/2."""
    import ml_dtypes
    lo = core * ROWS_PER_CORE
    hi = min(lo + ROWS_PER_CORE, ROWS_TOTAL)
    xr = np.zeros((ROWS_PER_CORE, C), dtype=np.uint16)
    xr[:hi - lo, :H] = xh16[lo:hi, 0::2]
    xr[:hi - lo, H:] = xh16[lo:hi, 1::2]
    return xr.reshape(N_TILES, 128, C).view(ml_dtypes.bfloat16)


def _build_nc(repeat: int = 1, passes_per_iter: int = 1):
    """One full pass over the core's shard; repeat>1 wraps it in a HW loop
    (used only for steady-state timing -- same data is reprocessed;
    passes_per_iter unrolls extra passes inside the loop body)."""
    import concourse.mybir as mybir
    import concourse.tile as tile
    from concourse import bacc

    fp32 = mybir.dt.float32
    bf16 = mybir.dt.bfloat16

    nc = bacc.Bacc("TRN2", target_bir_lowering=False)
    x_d = nc.dram_tensor("x", [N_TILES, 128, CP], bf16, kind="ExternalInput").ap()
    xr_d = nc.dram_tensor("xr", [N_TILES, 128, C], bf16, kind="ExternalInput").ap()
    w_d = nc.dram_tensor("w", [2 * KT, 128, H], bf16, kind="ExternalInput").ap()
    y_d = nc.dram_tensor("y", [N_TILES, 128, C], bf16, kind="ExternalOutput").ap()

    with tile.TileContext(nc) as tc:
        with (
            tc.tile_pool(name="wpool", bufs=1) as wpool,
            tc.tile_pool(name="io", bufs=8) as io,
            tc.tile_pool(name="psp", bufs=4, space="PSUM") as psp,
        ):
            # Per-k-tile W loads so the first matmul waits only for chunk 0.
            w_tiles = [wpool.tile([128, H], bf16, name=f"w{u}")
                       for u in range(2 * KT)]
            for u in range(2 * KT):
                nc.scalar.dma_start(out=w_tiles[u], in_=w_d[u])

            def one_pass():
                for t in range(N_TILES):
                    xt = io.tile([128, CP], bf16, tag="xt")
                    nc.sync.dma_start(out=xt, in_=x_d[t])
                    xr = io.tile([128, C], bf16, tag="xr")
                    nc.gpsimd.dma_start(out=xr, in_=xr_d[t])
                    ysb = io.tile([128, C], bf16, tag="y")
                    pss = []
                    for half in range(2):   # 0: y_even (x_odd@B1), 1: y_odd
                        ps = psp.tile([128, H], fp32, name=f"ps{half}",
                                      tag=f"ps{half}")
                        pss.append(ps)
                        for u in range(KT):
                            ku = half * KT + u
                            nc.tensor.matmul(
                                ps,
                                lhsT=xt[:, ku * 128:(ku + 1) * 128],
                                rhs=w_tiles[ku],
                                start=(u == 0),
                                stop=(u == KT - 1),
                            )
                    # + x/2 on DVE, one full-width in-place relu on ScalarE,
                    # output halves split across the SP and Pool DMA queues.
                    nc.vector.tensor_tensor(
                        ysb[:, :H], pss[0], xr[:, :H], mybir.AluOpType.add)
                    nc.vector.tensor_tensor(
                        ysb[:, H:], pss[1], xr[:, H:], mybir.AluOpType.add)
                    nc.scalar.activation(
                        ysb, ysb, mybir.ActivationFunctionType.Relu)
                    out_eng = nc.sync if t % 2 == 0 else nc.gpsimd
                    out_eng.dma_start(out=y_d[t], in_=ysb)

            if repeat == 1:
                one_pass()
            else:
                import concourse.mybir as _mb
                with tc.For_i(0, repeat, 1,
                              hint_engines=(_mb.EngineType.PE,),
                              staggered_reset=True):
                    for _ in range(passes_per_iter):
                        one_pass()
    nc.compile()
    return nc


def _make_in_maps(x: np.ndarray, scale: int):
    xf = np.ascontiguousarray(np.asarray(x, dtype=np.float32).reshape(-1, C))
    x16 = _f32_to_bf16_u16(xf)
    xh16 = _f32_to_bf16_u16(xf * 0.5)
    W = _build_w_parity(scale)
    return [{"x": _shard_xt(x16, i), "xr": _shard_xr(xh16, i), "w": W}
            for i in range(N_CORES)]


def kernel(x: np.ndarray, scale) -> np.ndarray:
    import sys
    if "/opt/trn_rl_repo" not in sys.path:
        sys.path.insert(0, "/opt/trn_rl_repo")
    from concourse.bass_utils import run_bass_kernel_spmd

    scale = int(np.asarray(scale))
    x = np.asarray(x, dtype=np.float32)
    orig_shape = x.shape

    if "nc" not in _CACHE:
        _CACHE["nc"] = _build_nc()
    nc = _CACHE["nc"]

    in_maps = _make_in_maps(x, scale)
    res = run_bass_kernel_spmd(nc, in_maps, list(range(N_CORES)))
    outs = []
    for i, r in enumerate(res.results):
        lo = i * ROWS_PER_CORE
        hi = min(lo + ROWS_PER_CORE, ROWS_TOTAL)
        yb = np.asarray(r["y"]).reshape(ROWS_PER_CORE, C)[:hi - lo]
        yf = _bf16_u16_to_f32(yb.view(np.uint16))
        yout = np.empty_like(yf)
        yout[:, 0::2] = yf[:, :H]   # un-permute parity ordering
        yout[:, 1::2] = yf[:, H:]
        outs.append(yout)
    y = np.concatenate(outs, axis=0).reshape(orig_shape)
    return y.astype(np.float32)


# revision 45
# speedup vs baseline: 1.3759x; 1.3395x over previous
"""HFreqC layer kernel for 8 Trainium2 NeuronCores.

The reference op (FFT -> zero centered low-freq band -> IFFT -> real -> relu)
is, up to the relu, a fixed real linear operator along the channel axis:
    y = x @ W,  W = Re(ifft(mask * fft(I)))^T   (728x728, symmetric circulant)

Key structure (scale=4 => the kept band is exactly half the spectrum): the
filter h(d) = (1/c) sum_{k in band} w^{kd} vanishes for all even lags d != 0
and h(0) = 1/2, so W couples only opposite parities plus a half-identity:
    y_even = relu(x_even/2 + x_odd  @ B1)   B1 = W[odd, even]  (364x364)
    y_odd  = relu(x_odd /2 + x_even @ B2)   B2 = W[even, odd]
This HALVES the matmul work vs the dense 728x728 GEMM.

Layout: pure data parallel over rows (46208 rows; 46 row-tiles of 128 per
core, last core zero-padded). All device I/O bf16 (~0.2% rel err vs the
2e-2 gate). Per row-tile:
  - [128, 768] bf16 channel-major DMA on SP: k-tiles 0-2 = odd channels
    (364 pad 384), 3-5 = even channels.
  - [128, 728] bf16 row-major x/2 DMA on GpSimd: [0:364]=x_even/2,
    [364:728]=x_odd/2 (host pre-scaled).
  - 3+3 bf16 matmuls accumulate B1/B2 products into two PSUM tiles.
  - DVE adds x/2 from SBUF onto each PSUM result (tensor_tensor) writing
    ysb in place; relu_even on ScalarE, relu_odd on DVE.
  - [128, 728] bf16 parity-ordered DMA out on ScalarE; host un-permutes.
Engine budget per row-tile (cost model): PE 6x364cyc ~ 910ns, DVE ~1140ns,
ACT ~1140ns, SP ~590ns, Pool ~560ns -> ~53us/core steady state.
"""

import numpy as np

C = 728            # channels
H = C // 2         # 364 per parity
KT = 3             # k-tiles of 128 per parity (364 pad 384)
CP = 2 * KT * 128  # 768 padded channels (odd block + even block)
N_CORES = 8
ROWS_TOTAL = 32 * 38 * 38          # 46208
N_TILES = 46                       # 128-row tiles per core
ROWS_PER_CORE = N_TILES * 128      # 5888 (padded; 8*5888 = 47104 >= 46208)

_CACHE = {}


def _f32_to_bf16_u16(a: np.ndarray) -> np.ndarray:
    """Round-to-nearest-even f32 -> bf16, as uint16 payload (fast, vectorized)."""
    u = np.ascontiguousarray(a).view(np.uint32)
    rounded = u + np.uint32(0x7FFF) + ((u >> np.uint32(16)) & np.uint32(1))
    return (rounded >> np.uint32(16)).astype(np.uint16)


def _bf16_u16_to_f32(u: np.ndarray) -> np.ndarray:
    return (u.astype(np.uint32) << np.uint32(16)).view(np.float32)


def _bf16(a: np.ndarray):
    import ml_dtypes
    return _f32_to_bf16_u16(np.ascontiguousarray(a)).view(ml_dtypes.bfloat16)


def _build_w(scale: int) -> np.ndarray:
    """Full [C, C] f32 W; y_row = x_row @ W."""
    m_sh = np.ones(C)
    m_sh[C // 2 - C // scale: C // 2 + C // scale] = 0
    m = np.fft.ifftshift(m_sh)
    A = np.fft.ifft(m[:, None] * np.fft.fft(np.eye(C), axis=0), axis=0)
    return np.real(A).T.astype(np.float32)


def _build_w_parity(scale: int) -> np.ndarray:
    """[2*KT, 128, H] bf16: tiles 0-2 = B1 = W[odd, even] rows (pad 384),
    tiles 3-5 = B2 = W[even, odd] rows. Asserts the parity structure."""
    W = _build_w(scale)
    d_e = W[0::2, 0::2]
    d_o = W[1::2, 1::2]
    assert np.abs(d_e - 0.5 * np.eye(H)).max() < 1e-5, "parity structure broken"
    assert np.abs(d_o - 0.5 * np.eye(H)).max() < 1e-5, "parity structure broken"
    B1 = W[1::2, 0::2]   # x_odd  -> y_even
    B2 = W[0::2, 1::2]   # x_even -> y_odd
    wp = np.zeros((2 * KT * 128, H), dtype=np.float32)
    wp[:H] = B1
    wp[KT * 128:KT * 128 + H] = B2
    return _bf16(wp.reshape(2 * KT, 128, H))


def _shard_xt(x16: np.ndarray, core: int) -> np.ndarray:
    """[N_TILES, 128, CP] bf16 channel-major: k-tiles 0-2 odd chs, 3-5 even.
    [t][p][u*128+m] = x[t*128+m, ch(u,p)]."""
    import ml_dtypes
    lo = core * ROWS_PER_CORE
    hi = min(lo + ROWS_PER_CORE, ROWS_TOTAL)
    xp = np.zeros((ROWS_PER_CORE, CP), dtype=np.uint16)
    xp[:hi - lo, :H] = x16[lo:hi, 1::2]            # odd channels
    xp[:hi - lo, KT * 128:KT * 128 + H] = x16[lo:hi, 0::2]  # even channels
    v = xp.reshape(N_TILES, 128, 2 * KT, 128)      # t m u p
    v = v.transpose(0, 3, 2, 1)                    # t p u m
    return np.ascontiguousarray(v).reshape(N_TILES, 128, CP).view(ml_dtypes.bfloat16)


def _build_nc(repeat: int = 1, passes_per_iter: int = 1):
    """One full pass over the core's shard; repeat>1 wraps it in a HW loop
    (used only for steady-state timing -- same data is reprocessed;
    passes_per_iter unrolls extra passes inside the loop body)."""
    import concourse.mybir as mybir
    import concourse.tile as tile
    from concourse import bacc

    fp32 = mybir.dt.float32
    bf16 = mybir.dt.bfloat16

    nc = bacc.Bacc("TRN2", target_bir_lowering=False)
    x_d = nc.dram_tensor("x", [N_TILES, 128, CP], bf16, kind="ExternalInput").ap()
    w_d = nc.dram_tensor("w", [2 * KT, 128, H], bf16, kind="ExternalInput").ap()
    id_d = nc.dram_tensor("ident", [128, 128], bf16, kind="ExternalInput").ap()
    y_d = nc.dram_tensor("y", [N_TILES, 128, C], bf16, kind="ExternalOutput").ap()

    with tile.TileContext(nc) as tc:
        with (
            tc.tile_pool(name="wpool", bufs=1) as wpool,
            tc.tile_pool(name="io", bufs=8) as io,
            tc.tile_pool(name="psp", bufs=4, space="PSUM") as psp,
        ):
            # Per-k-tile W loads so the first matmul waits only for chunk 0.
            # ident first: the opening seed matmuls need only it (+ xt).
            ident = wpool.tile([128, 128], bf16, name="ident")  # I/2
            nc.scalar.dma_start(out=ident, in_=id_d)
            w_tiles = [wpool.tile([128, H], bf16, name=f"w{u}")
                       for u in range(2 * KT)]
            for u in range(2 * KT):
                nc.scalar.dma_start(out=w_tiles[u], in_=w_d[u])

            # k-tile widths per parity: 128, 128, 108 (364 channels)
            kw = [128, 128, H - 256]

            def one_pass():
                for t in range(N_TILES):
                    xt = io.tile([128, CP], bf16, tag="xt")
                    nc.sync.dma_start(out=xt, in_=x_d[t])
                    ysb = io.tile([128, C], bf16, tag="y")
                    for half in range(2):   # 0: y_even, 1: y_odd
                        ps = psp.tile([128, H], fp32, name=f"ps{half}",
                                      tag=f"ps{half}")
                        # Seed ps with x_otherparity/2 by transposing the
                        # in-SBUF channel-major tiles through I/2 (rhs):
                        # out[r, c] = sum_k xt[k, r] * (I/2)[k, c].
                        for u in range(KT):
                            ku = (1 - half) * KT + u
                            w_ = kw[u]
                            nc.tensor.matmul(
                                ps[:, u * 128:u * 128 + w_],
                                lhsT=xt[:, ku * 128:(ku + 1) * 128],
                                rhs=ident[:, :w_],
                                start=(u == 0),
                                stop=False,
                                skip_group_check=True,
                            )
                        # Accumulate x_otherparity @ B on top.
                        for u in range(KT):
                            ku = half * KT + u
                            nc.tensor.matmul(
                                ps,
                                lhsT=xt[:, ku * 128:(ku + 1) * 128],
                                rhs=w_tiles[ku],
                                start=False,
                                stop=(u == KT - 1),
                                skip_group_check=True,
                            )
                        # relu: even half on ScalarE, odd half on DVE.
                        if half == 0:
                            nc.scalar.activation(
                                ysb[:, :H], ps,
                                mybir.ActivationFunctionType.Relu)
                        else:
                            nc.vector.tensor_scalar_max(ysb[:, H:], ps, 0.0)
                    out_eng = nc.scalar if t % 2 == 0 else nc.sync
                    out_eng.dma_start(out=y_d[t], in_=ysb)

            if repeat == 1:
                one_pass()
            else:
                import concourse.mybir as _mb
                with tc.For_i(0, repeat, 1,
                              hint_engines=(_mb.EngineType.PE,),
                              staggered_reset=True):
                    for _ in range(passes_per_iter):
                        one_pass()
    nc.compile()
    return nc


def _make_in_maps(x: np.ndarray, scale: int):
    xf = np.ascontiguousarray(np.asarray(x, dtype=np.float32).reshape(-1, C))
    x16 = _f32_to_bf16_u16(xf)
    W = _build_w_parity(scale)
    ident = _bf16(0.5 * np.eye(128, dtype=np.float32))
    return [{"x": _shard_xt(x16, i), "w": W, "ident": ident}
            for i in range(N_CORES)]


def kernel(x: np.ndarray, scale) -> np.ndarray:
    import sys
    if "/opt/trn_rl_repo" not in sys.path:
        sys.path.insert(0, "/opt/trn_rl_repo")
    from concourse.bass_utils import run_bass_kernel_spmd

    scale = int(np.asarray(scale))
    x = np.asarray(x, dtype=np.float32)
    orig_shape = x.shape

    if "nc" not in _CACHE:
        _CACHE["nc"] = _build_nc()
    nc = _CACHE["nc"]

    in_maps = _make_in_maps(x, scale)
    res = run_bass_kernel_spmd(nc, in_maps, list(range(N_CORES)))
    outs = []
    for i, r in enumerate(res.results):
        lo = i * ROWS_PER_CORE
        hi = min(lo + ROWS_PER_CORE, ROWS_TOTAL)
        yb = np.asarray(r["y"]).reshape(ROWS_PER_CORE, C)[:hi - lo]
        yf = _bf16_u16_to_f32(yb.view(np.uint16))
        yout = np.empty_like(yf)
        yout[:, 0::2] = yf[:, :H]   # un-permute parity ordering
        yout[:, 1::2] = yf[:, H:]
        outs.append(yout)
    y = np.concatenate(outs, axis=0).reshape(orig_shape)
    return y.astype(np.float32)


# revision 51
# speedup vs baseline: 1.5039x; 1.0931x over previous
"""HFreqC layer kernel for 8 Trainium2 NeuronCores.

The reference op (FFT -> zero centered low-freq band -> IFFT -> real -> relu)
is, up to the relu, a fixed real linear operator along the channel axis:
    y = x @ W,  W = Re(ifft(mask * fft(I)))^T   (728x728, symmetric circulant)

Key structure (scale=4 => the kept band is exactly half the spectrum): the
filter h(d) = (1/c) sum_{k in band} w^{kd} vanishes for all even lags d != 0
and h(0) = 1/2, so W couples only opposite parities plus a half-identity:
    y_even = relu(x_even/2 + x_odd  @ B1)   B1 = W[odd, even]  (364x364)
    y_odd  = relu(x_odd /2 + x_even @ B2)   B2 = W[even, odd]
This HALVES the matmul work vs the dense 728x728 GEMM.

Layout: pure data parallel over rows (46208 rows; 46 row-tiles of 128 per
core, last core zero-padded). All device I/O bf16 (~0.2% rel err vs the
2e-2 gate). Per row-tile:
  - [128, 768] bf16 channel-major DMA on SP: k-tiles 0-2 = odd channels
    (364 pad 384), 3-5 = even channels.
  - [128, 728] bf16 row-major x/2 DMA on GpSimd: [0:364]=x_even/2,
    [364:728]=x_odd/2 (host pre-scaled).
  - 3+3 bf16 matmuls accumulate B1/B2 products into two PSUM tiles.
  - DVE adds x/2 from SBUF onto each PSUM result (tensor_tensor) writing
    ysb in place; relu_even on ScalarE, relu_odd on DVE.
  - [128, 728] bf16 parity-ordered DMA out on ScalarE; host un-permutes.
Engine budget per row-tile (cost model): PE 6x364cyc ~ 910ns, DVE ~1140ns,
ACT ~1140ns, SP ~590ns, Pool ~560ns -> ~53us/core steady state.
"""

import numpy as np

C = 728            # channels
H = C // 2         # 364 per parity
KT = 3             # k-tiles of 128 per parity (364 pad 384)
CP = 2 * KT * 128  # 768 padded channels (odd block + even block)
N_CORES = 8
ROWS_TOTAL = 32 * 38 * 38          # 46208
N_TILES = 46                       # 128-row tiles per core
N_PAIRS = N_TILES // 2             # two row-tiles batched per DMA
ROWS_PER_CORE = N_TILES * 128      # 5888 (padded; 8*5888 = 47104 >= 46208)

_CACHE = {}


def _f32_to_bf16_u16(a: np.ndarray) -> np.ndarray:
    """Round-to-nearest-even f32 -> bf16, as uint16 payload (fast, vectorized)."""
    u = np.ascontiguousarray(a).view(np.uint32)
    rounded = u + np.uint32(0x7FFF) + ((u >> np.uint32(16)) & np.uint32(1))
    return (rounded >> np.uint32(16)).astype(np.uint16)


def _bf16_u16_to_f32(u: np.ndarray) -> np.ndarray:
    return (u.astype(np.uint32) << np.uint32(16)).view(np.float32)


def _bf16(a: np.ndarray):
    import ml_dtypes
    return _f32_to_bf16_u16(np.ascontiguousarray(a)).view(ml_dtypes.bfloat16)


def _build_w(scale: int) -> np.ndarray:
    """Full [C, C] f32 W; y_row = x_row @ W."""
    m_sh = np.ones(C)
    m_sh[C // 2 - C // scale: C // 2 + C // scale] = 0
    m = np.fft.ifftshift(m_sh)
    A = np.fft.ifft(m[:, None] * np.fft.fft(np.eye(C), axis=0), axis=0)
    return np.real(A).T.astype(np.float32)


def _build_w_parity(scale: int) -> np.ndarray:
    """[2*KT, 128, H] bf16: tiles 0-2 = B1 = W[odd, even] rows (pad 384),
    tiles 3-5 = B2 = W[even, odd] rows. Asserts the parity structure."""
    W = _build_w(scale)
    d_e = W[0::2, 0::2]
    d_o = W[1::2, 1::2]
    assert np.abs(d_e - 0.5 * np.eye(H)).max() < 1e-5, "parity structure broken"
    assert np.abs(d_o - 0.5 * np.eye(H)).max() < 1e-5, "parity structure broken"
    B1 = W[1::2, 0::2]   # x_odd  -> y_even
    B2 = W[0::2, 1::2]   # x_even -> y_odd
    wp = np.zeros((2 * KT * 128, H), dtype=np.float32)
    wp[:H] = B1
    wp[KT * 128:KT * 128 + H] = B2
    return _bf16(wp.reshape(2 * KT, 128, H))


def _shard_xt(x16: np.ndarray, core: int) -> np.ndarray:
    """[N_PAIRS, 128, 2*CP] bf16 channel-major, two row-tiles batched per
    DMA: [q][p][t2*CP + u*128 + m] = x[(2q+t2)*128+m, ch(u,p)] with k-tiles
    0-2 odd channels, 3-5 even channels."""
    import ml_dtypes
    lo = core * ROWS_PER_CORE
    hi = min(lo + ROWS_PER_CORE, ROWS_TOTAL)
    xp = np.zeros((ROWS_PER_CORE, CP), dtype=np.uint16)
    xp[:hi - lo, :H] = x16[lo:hi, 1::2]            # odd channels
    xp[:hi - lo, KT * 128:KT * 128 + H] = x16[lo:hi, 0::2]  # even channels
    v = xp.reshape(N_PAIRS, 2, 128, 2 * KT, 128)   # q t2 m u p
    v = v.transpose(0, 4, 1, 3, 2)                 # q p t2 u m
    return np.ascontiguousarray(v).reshape(N_PAIRS, 128, 2 * CP).view(ml_dtypes.bfloat16)


def _build_nc(repeat: int = 1, passes_per_iter: int = 1):
    """One full pass over the core's shard; repeat>1 wraps it in a HW loop
    (used only for steady-state timing -- same data is reprocessed;
    passes_per_iter unrolls extra passes inside the loop body)."""
    import concourse.mybir as mybir
    import concourse.tile as tile
    from concourse import bacc

    fp32 = mybir.dt.float32
    bf16 = mybir.dt.bfloat16

    nc = bacc.Bacc("TRN2", target_bir_lowering=False)
    x_d = nc.dram_tensor("x", [N_PAIRS, 128, 2 * CP], bf16,
                         kind="ExternalInput").ap()
    w_d = nc.dram_tensor("w", [2 * KT, 128, H], bf16, kind="ExternalInput").ap()
    id_d = nc.dram_tensor("ident", [128, 128], bf16, kind="ExternalInput").ap()
    y_d = nc.dram_tensor("y", [N_PAIRS, 128, 2 * C], bf16,
                         kind="ExternalOutput").ap()

    with tile.TileContext(nc) as tc:
        with (
            tc.tile_pool(name="wpool", bufs=1) as wpool,
            tc.tile_pool(name="io", bufs=8) as io,
            tc.tile_pool(name="psp", bufs=4, space="PSUM") as psp,
        ):
            # Per-k-tile W loads so the first matmul waits only for chunk 0.
            # ident first: the opening seed matmuls need only it (+ xt).
            ident = wpool.tile([128, 128], bf16, name="ident")  # I/2
            nc.scalar.dma_start(out=ident, in_=id_d)
            w_tiles = [wpool.tile([128, H], bf16, name=f"w{u}")
                       for u in range(2 * KT)]
            for u in range(2 * KT):
                nc.scalar.dma_start(out=w_tiles[u], in_=w_d[u])

            # k-tile widths per parity: 128, 128, 108 (364 channels)
            kw = [128, 128, H - 256]

            def one_pass():
                for q in range(N_PAIRS):
                    xt = io.tile([128, 2 * CP], bf16, tag="xt")
                    nc.sync.dma_start(out=xt, in_=x_d[q])
                    ysb = io.tile([128, 2 * C], bf16, tag="y")
                    for t2 in range(2):
                        x0 = t2 * CP
                        y0 = t2 * C
                        for half in range(2):   # 0: y_even, 1: y_odd
                            ps = psp.tile([128, H], fp32, name=f"ps{half}",
                                          tag=f"ps{half}")
                            # Seed ps with x_otherparity/2 by transposing the
                            # in-SBUF channel-major tiles through I/2 (rhs):
                            # out[r, c] = sum_k xt[k, r] * (I/2)[k, c].
                            for u in range(KT):
                                ku = x0 + ((1 - half) * KT + u) * 128
                                w_ = kw[u]
                                nc.tensor.matmul(
                                    ps[:, u * 128:u * 128 + w_],
                                    lhsT=xt[:, ku:ku + 128],
                                    rhs=ident[:, :w_],
                                    start=(u == 0),
                                    stop=False,
                                    skip_group_check=True,
                                )
                            # Accumulate x_otherparity @ B on top.
                            for u in range(KT):
                                ku = x0 + (half * KT + u) * 128
                                nc.tensor.matmul(
                                    ps,
                                    lhsT=xt[:, ku:ku + 128],
                                    rhs=w_tiles[half * KT + u],
                                    start=False,
                                    stop=(u == KT - 1),
                                    skip_group_check=True,
                                )
                            # relu: even half on ScalarE, odd half on DVE.
                            j0 = y0 + half * H
                            if half == 0:
                                nc.scalar.activation(
                                    ysb[:, j0:j0 + H], ps,
                                    mybir.ActivationFunctionType.Relu)
                            else:
                                nc.vector.tensor_scalar_max(
                                    ysb[:, j0:j0 + H], ps, 0.0)
                    out_eng = nc.scalar if q % 2 == 0 else nc.sync
                    out_eng.dma_start(out=y_d[q], in_=ysb)

            if repeat == 1:
                one_pass()
            else:
                import concourse.mybir as _mb
                with tc.For_i(0, repeat, 1,
                              hint_engines=(_mb.EngineType.PE,),
                              staggered_reset=True):
                    for _ in range(passes_per_iter):
                        one_pass()
    nc.compile()
    return nc


def _make_in_maps(x: np.ndarray, scale: int):
    xf = np.ascontiguousarray(np.asarray(x, dtype=np.float32).reshape(-1, C))
    x16 = _f32_to_bf16_u16(xf)
    W = _build_w_parity(scale)
    ident = _bf16(0.5 * np.eye(128, dtype=np.float32))
    return [{"x": _shard_xt(x16, i), "w": W, "ident": ident}
            for i in range(N_CORES)]


def kernel(x: np.ndarray, scale) -> np.ndarray:
    import sys
    if "/opt/trn_rl_repo" not in sys.path:
        sys.path.insert(0, "/opt/trn_rl_repo")
    from concourse.bass_utils import run_bass_kernel_spmd

    scale = int(np.asarray(scale))
    x = np.asarray(x, dtype=np.float32)
    orig_shape = x.shape

    if "nc" not in _CACHE:
        _CACHE["nc"] = _build_nc()
    nc = _CACHE["nc"]

    in_maps = _make_in_maps(x, scale)
    res = run_bass_kernel_spmd(nc, in_maps, list(range(N_CORES)))
    outs = []
    for i, r in enumerate(res.results):
        lo = i * ROWS_PER_CORE
        hi = min(lo + ROWS_PER_CORE, ROWS_TOTAL)
        yb = np.asarray(r["y"]).reshape(N_PAIRS, 128, 2, C)   # q p t2 j
        yb = yb.transpose(0, 2, 1, 3).reshape(ROWS_PER_CORE, C)[:hi - lo]
        yf = _bf16_u16_to_f32(np.ascontiguousarray(yb).view(np.uint16))
        yout = np.empty_like(yf)
        yout[:, 0::2] = yf[:, :H]   # un-permute parity ordering
        yout[:, 1::2] = yf[:, H:]
        outs.append(yout)
    y = np.concatenate(outs, axis=0).reshape(orig_shape)
    return y.astype(np.float32)
